# revision 23
# baseline (speedup 1.0000x reference)
"""Multi-scale deformable attention TRN2 kernel.

Sharding: data-parallel over batch B=8, one batch element per NeuronCore.

Per-core pipeline:
  P1: value projection v = value @ W_v (+b_v), written to an HBM table
      [NH*N, HD] in fp16 (row = one head's 32-dim feature of one token).
  P2: query projections (offsets, attention logits), sampling-location math,
      bilinear corner weights (folded with unnormalized softmax weights) and
      int32 gather indices, all in q-partition-major layout.
  P3: 16 indirect-DMA gathers of row PAIRS (x0,x0+1) per (sample, y-corner);
      DVE weight-mults; PE identity-matmul accumulation over (level, point,
      y) into PSUM.
  P4: normalize by softmax denominator, output projection, DMA out.
"""

import math
import sys

sys.path.insert(0, "/opt/trn_rl_repo")

import numpy as np

import concourse.bass as bass
import concourse.bacc as bacc
import concourse.mybir as mybir
import concourse.tile as tile
from concourse.bass_utils import run_bass_kernel_spmd
from concourse.masks import make_identity

F32 = mybir.dt.float32
F16 = mybir.dt.float16
I32 = mybir.dt.int32
I16 = mybir.dt.int16
AF = mybir.ActivationFunctionType
OP = mybir.AluOpType

D, NH, HD, L, P = 256, 8, 32, 4, 4
SS = [(128, 128), (64, 64), (32, 32), (16, 16)]  # (H, W)
LSI = [0, 16384, 20480, 21504]
N, B, Q = 21760, 8, 1000
NT = N // 128            # 170 value chunks
QB = 8                   # query blocks of 128 (Q padded to 1024)
QTAIL = Q - 7 * 128      # 104
NROWS = NH * N           # gather-table rows

_CACHE = {}


def _build_module():
    nc = bacc.Bacc()

    # ---- DRAM I/O ----
    value = nc.dram_tensor("value", [N, D], F32, kind="ExternalInput")
    query = nc.dram_tensor("query", [Q, D], F32, kind="ExternalInput")
    refp = nc.dram_tensor("refp", [Q, 2], F32, kind="ExternalInput")
    wv = nc.dram_tensor("wv", [D, D], F32, kind="ExternalInput")
    woff = nc.dram_tensor("woff", [D, D], F32, kind="ExternalInput")
    watt = nc.dram_tensor("watt", [D, NH * L * P], F32, kind="ExternalInput")
    wout = nc.dram_tensor("wout", [D, D], F32, kind="ExternalInput")
    bv = nc.dram_tensor("bv", [D], F32, kind="ExternalInput")
    boff = nc.dram_tensor("boff", [D], F32, kind="ExternalInput")
    batt = nc.dram_tensor("batt", [NH * L * P], F32, kind="ExternalInput")
    bout = nc.dram_tensor("bout", [D], F32, kind="ExternalInput")
    # hconst[p, l*8+h] = h*N + LSI[l]  (same for every partition p)
    hconst = nc.dram_tensor("hconst", [128, L * NH], F32, kind="ExternalInput")
    out = nc.dram_tensor("out", [Q, D], F32, kind="ExternalOutput")

    with tile.TileContext(nc) as tc:
        with (
            tc.tile_pool(name="const", bufs=1) as cpool,
            tc.tile_pool(name="work", bufs=3) as wpool,
            tc.tile_pool(name="qmath", bufs=1) as qpool,
            tc.tile_pool(name="gath", bufs=2) as gpool,
            tc.tile_pool(name="mtile", bufs=2) as mpool,
            tc.tile_pool(name="psum", bufs=2, space="PSUM") as pp,
            tc.tile_pool(name="psum_acc", bufs=1, space="PSUM") as pacc,
            tc.tile_pool(name="dram", bufs=1, space="DRAM") as dpool,
        ):
            # ---------- constants ----------
            ident = cpool.tile([128, 128], F32)
            make_identity(nc, ident[:])
            ident_h = cpool.tile([128, 128], F16)
            nc.vector.tensor_copy(ident_h[:], ident[:])
            wv_t = cpool.tile([128, 2, D], F32)
            nc.sync.dma_start(wv_t[:], wv.rearrange("(ko ki) n -> ki ko n", ki=128))
            woff_t = cpool.tile([128, 2, D], F32)
            nc.sync.dma_start(woff_t[:], woff.rearrange("(ko ki) n -> ki ko n", ki=128))
            watt_t = cpool.tile([128, 2, 128], F32)
            nc.sync.dma_start(watt_t[:], watt.rearrange("(ko ki) n -> ki ko n", ki=128))
            wout_t = cpool.tile([128, 2, D], F32)
            nc.sync.dma_start(wout_t[:], wout.rearrange("(ko ki) n -> ki ko n", ki=128))
            # rank-1 bias helpers: lhsT with row0 = ones; rhs tiles with row0 = bias
            ones_row = cpool.tile([128, 128], F32)
            nc.vector.memset(ones_row[:], 0.0)
            nc.vector.memset(ones_row[:1, :], 1.0)
            bv_row = cpool.tile([128, D], F32)
            nc.vector.memset(bv_row[:], 0.0)
            nc.sync.dma_start(bv_row[:1, :], bv[None, :])
            boff_row = cpool.tile([128, D], F32)
            nc.vector.memset(boff_row[:], 0.0)
            nc.sync.dma_start(boff_row[:1, :], boff[None, :])
            batt_row = cpool.tile([128, 128], F32)
            nc.vector.memset(batt_row[:], 0.0)
            nc.sync.dma_start(batt_row[:1, :], batt[None, :])
            bout_row = cpool.tile([128, D], F32)
            nc.vector.memset(bout_row[:], 0.0)
            nc.sync.dma_start(bout_row[:1, :], bout[None, :])
            hc = cpool.tile([128, L * NH], F32)
            nc.sync.dma_start(hc[:], hconst[:, :])

            # gather table in DRAM, fp32 shingled: row r = [v_r | v_{r+1}]
            vtab = dpool.tile([NROWS, 2 * HD], F32)
            # per-level int16 gather-index scratch (DRAM) + wrapped tiles
            ixs_dram = [dpool.tile([128, NH * P * 2 * QB], I16, name=f"ixd_{l}")
                        for l in range(L)]

            tc.strict_bb_all_engine_barrier()

            # ---------- P1: value projection ----------
            for nt in range(NT):
                vc = wpool.tile([128, D], F32, tag="vc")
                nc.sync.dma_start(vc[:], value[nt * 128:(nt + 1) * 128, :])
                vcT = wpool.tile([128, 2, 128], F32, tag="vcT")
                for k in range(2):
                    pt = pp.tile([128, 128], F32, tag="pt_tr")
                    nc.tensor.transpose(pt[:], vc[:, k * 128:(k + 1) * 128], ident[:])
                    nc.scalar.activation(vcT[:, k, :], pt[:], AF.Copy)
                pv = pp.tile([128, D], F32, tag="pmm")
                nc.tensor.matmul(pv[:], vcT[:, 0, :], wv_t[:, 0, :],
                                 start=True, stop=False)
                nc.tensor.matmul(pv[:], vcT[:, 1, :], wv_t[:, 1, :],
                                 start=False, stop=False)
                nc.tensor.matmul(pv[:], ones_row[:], bv_row[:],
                                 start=False, stop=True)
                vh = wpool.tile([128, D], F32, tag="vh")
                nc.scalar.activation(vh[:], pv[:], AF.Copy)
                # shingled table writes: row r cols0:32 = v_r ; cols32:64 = v_{r+1}
                vt3 = vtab[:].rearrange("(h n) d -> h n d", h=NH)
                dst1 = vt3[:, nt * 128:(nt + 1) * 128, 0:HD].rearrange(
                    "h n d -> n h d")
                nc.sync.dma_start(dst1, vh[:].rearrange("n (h d) -> n h d", h=NH))
                if nt == 0:
                    dst2 = vt3[:, 0:127, HD:2 * HD].rearrange("h n d -> n h d")
                    nc.sync.dma_start(
                        dst2, vh[1:, :].rearrange("n (h d) -> n h d", h=NH))
                else:
                    dst2 = vt3[:, nt * 128 - 1:(nt + 1) * 128 - 1,
                               HD:2 * HD].rearrange("h n d -> n h d")
                    nc.sync.dma_start(
                        dst2, vh[:].rearrange("n (h d) -> n h d", h=NH))

                if nt == NT - 1:
                    dstl = vt3[:, N - 1:N, HD:2 * HD].rearrange("h n d -> n h d")
                    nc.sync.dma_start(
                        dstl, vh[127:, :].rearrange("n (h d) -> n h d", h=NH))

            tc.strict_bb_all_engine_barrier()

            # ---------- P2a: query projections ----------
            off_sb = qpool.tile([128, QB, D], F32)       # c=(h,l,p,xy)
            e_sb = qpool.tile([128, QB, 128], F32)       # c2=(h,l,p)
            zbuf = qpool.tile([128, QB, NH], F32)
            rz = qpool.tile([128, QB, NH], F32)
            refb = qpool.tile([128, QB, 2], F32)
            nc.vector.memset(refb[:], 0.0)
            nc.sync.dma_start(
                refb[:, :7, :],
                refp[: 7 * 128, :].rearrange("(qb p) c -> p qb c", p=128),
            )
            nc.sync.dma_start(
                refb[:QTAIL, 7, :],
                refp[7 * 128:, :],
            )
            for qb in range(QB):
                qc = wpool.tile([128, D], F32, tag="qc")
                if qb == 7:
                    nc.vector.memset(qc[:], 0.0)
                    nc.sync.dma_start(qc[:QTAIL, :], query[qb * 128:, :])
                else:
                    nc.sync.dma_start(qc[:], query[qb * 128:(qb + 1) * 128, :])
                qT = wpool.tile([128, 2, 128], F32, tag="qT")
                for k in range(2):
                    pt = pp.tile([128, 128], F32, tag="pt_tr")
                    nc.tensor.transpose(pt[:], qc[:, k * 128:(k + 1) * 128], ident[:])
                    nc.scalar.activation(qT[:, k, :], pt[:], AF.Copy)
                po = pp.tile([128, D], F32, tag="pmm")
                nc.tensor.matmul(po[:], qT[:, 0, :], woff_t[:, 0, :],
                                 start=True, stop=False)
                nc.tensor.matmul(po[:], qT[:, 1, :], woff_t[:, 1, :],
                                 start=False, stop=False)
                nc.tensor.matmul(po[:], ones_row[:], boff_row[:],
                                 start=False, stop=True)
                nc.scalar.activation(off_sb[:, qb, :], po[:], AF.Copy)
                pa = pp.tile([128, D], F32, tag="pmm", name="pa")
                nc.tensor.matmul(pa[:, :128], qT[:, 0, :], watt_t[:, 0, :],
                                 start=True, stop=False)
                nc.tensor.matmul(pa[:, :128], qT[:, 1, :], watt_t[:, 1, :],
                                 start=False, stop=False)
                nc.tensor.matmul(pa[:, :128], ones_row[:], batt_row[:],
                                 start=False, stop=True)
                nc.scalar.activation(e_sb[:, qb, :], pa[:, :128], AF.Exp)
                nc.vector.tensor_reduce(
                    zbuf[:, qb, :],
                    e_sb[:, qb, :].rearrange("p (h s) -> p h s", h=NH),
                    axis=mybir.AxisListType.X,
                    op=OP.add,
                )
            nc.vector.reciprocal(rz[:], zbuf[:])

            # ---------- P2b: sampling locations, weights, indices ----------
            # per level: W4buf [128, h, p, y, qb, x] f32 ; idx32 [128, h, p, y, qb]
            w4 = [qpool.tile([128, NH, P, 2, QB, 2], F32, tag=f"w4_{l}", name=f"w4_{l}")
                  for l in range(L)]
            idx16 = [qpool.tile([128, NH, P, 2, QB], I16, tag=f"ix_{l}", name=f"ix_{l}")
                     for l in range(L)]
            SH = [128, QB, NH, P]

            def lerp_side(l, xy, dimsz):
                """returns (frac, slot-w0, slot-w1, clipped corner0 f32, A-side)"""
                w = float(dimsz)
                o = off_sb[:, :, :].rearrange(
                    "pp qb (h l p c) -> pp qb h l p c", h=NH, l=L, p=P
                )[:, :, :, l, :, xy]
                x = qpool.tile(SH, F32, tag="t_x")
                nc.vector.tensor_scalar(x[:], o, 1.0 / (w + 1e-6), None, OP.mult)
                rb = refb[:, :, xy][:, :, None, None]
                nc.vector.tensor_tensor(x[:], x[:], rb.to_broadcast(SH), OP.add)
                nc.vector.tensor_scalar(x[:], x[:], 0.0, 1.0, OP.max, OP.min)
                nc.vector.tensor_scalar(x[:], x[:], w, -0.5, OP.mult, OP.add)
                # floor via truncation (x+1 >= 0.5 > 0)
                t = qpool.tile(SH, F32, tag="t_t")
                nc.vector.tensor_scalar(t[:], x[:], 1.0, None, OP.add)
                ti = qpool.tile(SH, I32, tag="t_ti")
                nc.vector.tensor_copy(ti[:], t[:])
                nc.vector.tensor_copy(t[:], ti[:])
                x0 = qpool.tile(SH, F32, tag="t_x0")
                nc.vector.tensor_scalar(x0[:], t[:], -1.0, None, OP.add)
                # robust to either trunc or round-to-nearest int casts:
                # if candidate > x, subtract 1
                cgt = qpool.tile(SH, F32, tag="t_cgt")
                nc.vector.tensor_tensor(cgt[:], x0[:], x[:], OP.is_gt)
                nc.vector.tensor_tensor(x0[:], x0[:], cgt[:], OP.subtract)
                fx = qpool.tile(SH, F32, tag="t_fx")
                nc.vector.tensor_tensor(fx[:], x[:], x0[:], OP.subtract)
                a = qpool.tile(SH, F32, tag="t_a")
                nc.vector.tensor_scalar(a[:], x0[:], 0.0, None, OP.is_ge)
                b_ = qpool.tile(SH, F32, tag="t_b")
                nc.vector.tensor_scalar(b_[:], x0[:], w - 2.0, None, OP.is_le)
                return x0, fx, a, b_

            for l in range(L):
                H, W = SS[l]
                x0, fx, ax, bx = lerp_side(l, 0, W)
                # slot weights along x
                ab = qpool.tile(SH, F32, tag="t_ab")
                nc.vector.tensor_tensor(ab[:], ax[:], bx[:], OP.mult)
                fxc = qpool.tile(SH, F32, tag="t_fxc")
                nc.vector.tensor_scalar(fxc[:], fx[:], -1.0, 1.0, OP.mult, OP.add)
                na = qpool.tile(SH, F32, tag="t_na")
                nc.vector.tensor_scalar(na[:], ax[:], -1.0, 1.0, OP.mult, OP.add)
                nb = qpool.tile(SH, F32, tag="t_nb")
                nc.vector.tensor_scalar(nb[:], bx[:], -1.0, 1.0, OP.mult, OP.add)
                sw0 = qpool.tile(SH, F32, tag="t_sw0")
                sw1 = qpool.tile(SH, F32, tag="t_sw1")
                tmp = qpool.tile(SH, F32, tag="t_tmp")
                nc.vector.tensor_tensor(sw0[:], ab[:], fxc[:], OP.mult)
                nc.vector.tensor_tensor(tmp[:], na[:], fx[:], OP.mult)
                nc.vector.tensor_tensor(sw0[:], sw0[:], tmp[:], OP.add)
                nc.vector.tensor_tensor(sw1[:], ab[:], fx[:], OP.mult)
                nc.vector.tensor_tensor(tmp[:], nb[:], fxc[:], OP.mult)
                nc.vector.tensor_tensor(sw1[:], sw1[:], tmp[:], OP.add)
                xs = qpool.tile(SH, F32, tag="t_xs")
                nc.vector.tensor_scalar(xs[:], x0[:], 0.0, float(W - 2), OP.max, OP.min)

                y0, fy, ay, by = lerp_side(l, 1, H)
                fyc = qpool.tile(SH, F32, tag="t_fyc")
                nc.vector.tensor_scalar(fyc[:], fy[:], -1.0, 1.0, OP.mult, OP.add)
                wy0 = qpool.tile(SH, F32, tag="t_wy0")
                nc.vector.tensor_tensor(wy0[:], fyc[:], ay[:], OP.mult)
                wy1 = qpool.tile(SH, F32, tag="t_wy1")
                nc.vector.tensor_tensor(wy1[:], fy[:], by[:], OP.mult)
                ys0 = qpool.tile(SH, F32, tag="t_ys0")
                nc.vector.tensor_scalar(ys0[:], y0[:], 0.0, float(H - 1), OP.max, OP.min)
                ys1 = qpool.tile(SH, F32, tag="t_ys1")
                nc.vector.tensor_scalar(ys1[:], y0[:], 1.0, None, OP.add)
                nc.vector.tensor_scalar(ys1[:], ys1[:], 0.0, float(H - 1), OP.max, OP.min)

                el = e_sb[:, :, :].rearrange(
                    "pp qb (h l p) -> pp qb h l p", h=NH, l=L
                )[:, :, :, l, :]
                ty0 = qpool.tile(SH, F32, tag="t_ty0")
                nc.vector.tensor_tensor(ty0[:], wy0[:], el, OP.mult)
                ty1 = qpool.tile(SH, F32, tag="t_ty1")
                nc.vector.tensor_tensor(ty1[:], wy1[:], el, OP.mult)
                for y, ty in ((0, ty0), (1, ty1)):
                    for x_, sw in ((0, sw0), (1, sw1)):
                        dstw = w4[l][:, :, :, y, :, x_].rearrange(
                            "pp h p qb -> pp qb h p"
                        )
                        nc.vector.tensor_tensor(dstw, ty[:], sw[:], OP.mult)
                for y, ys in ((0, ys0), (1, ys1)):
                    idf = qpool.tile(SH, F32, tag="t_idf")
                    nc.vector.tensor_scalar(idf[:], ys[:], float(W), None, OP.mult)
                    nc.vector.tensor_tensor(idf[:], idf[:], xs[:], OP.add)
                    dsti = idx16[l][:, :, :, y, :].rearrange("pp h p qb -> pp qb h p")
                    nc.vector.tensor_copy(dsti, idf[:])

            tc.strict_bb_all_engine_barrier()

            # ---------- P3: gather + weighted accumulate ----------
            # re-layout indices per level into dma_gather wrapped form:
            #   sample i = c*128 + q128  (c = (h,p,y,qb) 512 cols)
            #   idx value at [i%16, i//16] ; replicated over 8 core groups
            acc = [pacc.tile([128, 2 * QB * HD], F32, tag=f"acc{hp}", name=f"acc{hp}")
                   for hp in range(4)]
            vt3 = vtab[:].rearrange("(h n) d -> h n d", h=NH)
            CPL = NH * P * 2 * QB          # 512 idx cols per level
            idxw = [qpool.tile([128, CPL * 8], I16, tag=f"ixw_{l}",
                               name=f"ixw_{l}") for l in range(L)]
            for l in range(L):
                # hop1: [128 q, 512 cols] -> DRAM (row-major per partition)
                nc.sync.dma_start(
                    ixs_dram[l][:, :],
                    idx16[l][:, :, :, :, :].rearrange("pp h p y qb -> pp (h p y qb)"),
                )
                # hop2: wrapped layout [16, (hc 512, qhi 8)] x 8 replicas
                sflat = ixs_dram[l][:, :].rearrange("p c -> (p c)")
                for rep in range(8):
                    dst = idxw[l][rep * 16:(rep + 1) * 16, :].rearrange(
                        "p (hc qhi) -> p hc qhi", qhi=8)
                    srcv = sflat.rearrange(
                        "(qhi q16 hc) -> q16 hc qhi", q16=16, hc=CPL)
                    nc.sync.dma_start(dst, srcv)
            _staged = tc.strict_bb_all_engine_barrier()
            for h in range(NH):
                hp, hl = h // 2, h % 2
                for l in range(L):
                    H, W = SS[l]
                    HW = H * W
                    inap = vt3[h, LSI[l]:LSI[l] + HW, :]
                    g = gpool.tile([128, P * 2 * QB, 2 * HD], F32, tag="g")
                    nc.gpsimd.dma_gather(
                        out_ap=g[:],
                        in_ap=inap,
                        idxs_ap=idxw[l][:, h * (CPL // NH) * 8:
                                        (h + 1) * (CPL // NH) * 8],
                        num_idxs=P * 2 * QB * 128,
                        num_idxs_reg=P * 2 * QB * 128,
                        elem_size=2 * HD,
                        elem_step=2 * HD,
                        single_packet=False,
                    )
                    for x_ in range(2):
                        m = mpool.tile([128, P, 2, QB, HD], F16, tag=f"m{x_}")
                        wslice = w4[l][:, h, :, :, :, x_].rearrange(
                            "pp p y qb -> pp (p y qb)")[:, :, None]
                        gs = g[:, :, x_ * HD:(x_ + 1) * HD]
                        nc.vector.tensor_tensor(
                            m[:].rearrange("pp p y qb d -> pp (p y qb) d"),
                            gs,
                            wslice.to_broadcast([128, P * 2 * QB, HD]),
                            OP.mult,
                        )
                        for pi in range(P):
                            for y in range(2):
                                rhs = m[:, pi, y, :, :]
                                first = (l == 0 and x_ == 0 and pi == 0 and y == 0)
                                last = (l == L - 1 and x_ == 1 and pi == P - 1
                                        and y == 1)
                                nc.tensor.matmul(
                                    acc[hp][:, hl * QB * HD:(hl + 1) * QB * HD],
                                    ident_h[:], rhs,
                                    start=first, stop=last,
                                )

            tc.strict_bb_all_engine_barrier()

            # ---------- P4: normalize + output projection ----------
            for qb in range(QB):
                mn = wpool.tile([128, D], F32, tag="mn")  # [q, (h hd)]
                for hp in range(4):
                    src = acc[hp][:].rearrange(
                        "pp (h qb d) -> pp h qb d", h=2, qb=QB
                    )[:, :, qb, :]
                    rzs = rz[:, qb, 2 * hp:2 * hp + 2][:, :, None]
                    nc.vector.tensor_tensor(
                        mn[:, 2 * hp * HD:(2 * hp + 2) * HD].rearrange(
                            "pp (h d) -> pp h d", h=2
                        ),
                        src,
                        rzs.to_broadcast([128, 2, HD]),
                        OP.mult,
                    )
                mT = wpool.tile([128, 2, 128], F32, tag="mT")
                for k in range(2):
                    pt = pp.tile([128, 128], F32, tag="pt_tr")
                    nc.tensor.transpose(pt[:], mn[:, k * 128:(k + 1) * 128], ident[:])
                    nc.scalar.activation(mT[:, k, :], pt[:], AF.Copy)
                pout = pp.tile([128, D], F32, tag="pmm")
                nc.tensor.matmul(pout[:], mT[:, 0, :], wout_t[:, 0, :],
                                 start=True, stop=False)
                nc.tensor.matmul(pout[:], mT[:, 1, :], wout_t[:, 1, :],
                                 start=False, stop=False)
                nc.tensor.matmul(pout[:], ones_row[:], bout_row[:],
                                 start=False, stop=True)
                osb = wpool.tile([128, D], F32, tag="osb")
                nc.scalar.activation(osb[:], pout[:], AF.Copy)
                if qb == 7:
                    nc.sync.dma_start(out[qb * 128:, :], osb[:QTAIL, :])
                else:
                    nc.sync.dma_start(out[qb * 128:(qb + 1) * 128, :], osb[:])
    nc.finalize()
    return nc


def _get_module():
    if "nc" not in _CACHE:
        _CACHE["nc"] = _build_module()
    return _CACHE["nc"]


def kernel(**inputs):
    nc = _get_module()
    q = np.asarray(inputs["query"], np.float32)        # [Q, B, D]
    ref = np.asarray(inputs["reference_points"], np.float32)
    val = np.asarray(inputs["value"], np.float32)      # [N, B, D]
    hconst = np.zeros((128, L * NH), np.float32)
    for l in range(L):
        for h in range(NH):
            hconst[:, l * NH + h] = h * N + LSI[l]
    common = {
        "wv": np.ascontiguousarray(inputs["W_v"], np.float32),
        "woff": np.ascontiguousarray(inputs["W_off"], np.float32),
        "watt": np.ascontiguousarray(inputs["W_attn"], np.float32),
        "wout": np.ascontiguousarray(inputs["W_out"], np.float32),
        "bv": np.ascontiguousarray(inputs["b_v"], np.float32),
        "boff": np.ascontiguousarray(inputs["b_off"], np.float32),
        "batt": np.ascontiguousarray(inputs["b_attn"], np.float32),
        "bout": np.ascontiguousarray(inputs["b_out"], np.float32),
        "hconst": hconst,
    }
    in_maps = []
    for b in range(B):
        m = dict(common)
        m["value"] = np.ascontiguousarray(val[:, b, :])
        m["query"] = np.ascontiguousarray(q[:, b, :])
        m["refp"] = np.ascontiguousarray(ref[:, b, :])
        in_maps.append(m)
    res = run_bass_kernel_spmd(nc, in_maps, list(range(B)))
    outs = [r["out"] for r in res.results]
    return np.stack(outs, axis=1)  # [Q, B, D]
